# revision 9
# baseline (speedup 1.0000x reference)
"""GQA attention layer (RMSNorm QK + RoPE + causal SDPA + out-proj) on 8 TRN2 cores.

Sharding: 8 cores = 2 (batch) x 4 (kv-head groups). Each core computes, for its
batch b and kv group g: q/k/v projections for heads 4g..4g+3 (+kv head g),
per-head RMSNorm + RoPE, causal attention, and a partial output projection
(attn_g @ Wo_g.T).  Host sums the 4 partial outputs per batch (TP unshard) and
stacks k/v.
"""
import sys

sys.path.insert(0, "/opt/trn_rl_repo")

import numpy as np

import concourse.bass as bass
import concourse.mybir as mybir
import concourse.tile as tile
from concourse import bacc
from concourse.bass_utils import run_bass_kernel_spmd
from concourse.masks import make_identity

# Problem constants (hardcoded per contract)
B, S, D = 2, 2048, 1536
H, KV, Dh = 16, 4, 96
HG = H // KV          # 4 q-heads per kv group
QW = HG * Dh          # 384 q cols per core
EPS = 1e-6
SCALING = Dh ** -0.5
N_CORES = 8
S16 = S // 128        # 16 s-blocks of 128
QB = S // 512         # 4 q-blocks of 512
KB = S // 128         # 16 k-blocks of 128

F32 = mybir.dt.float32
F32R = mybir.dt.float32r
AF = mybir.ActivationFunctionType
ALU = mybir.AluOpType

# Per-stage matmul dtype: float32r runs the PE at full rate (vs 4x slower f32)
# with ~tf32-grade rounding.  Toggle to F32 per stage if precision demands.
DT_QKV = F32R
DT_SCORES = F32R
DT_PV = F32R
DT_OUT = F32R


def _mm_ops(a, b, dt_):
    if dt_ == F32R:
        return a.bitcast(F32R), b.bitcast(F32R)
    return a, b


def build_program():
    nc = bacc.Bacc("TRN2", target_bir_lowering=False, debug=False,
                   num_devices=N_CORES)

    hsT_d = nc.dram_tensor("hsT", [D, S], F32R, kind="ExternalInput").ap()
    wqkvT_d = nc.dram_tensor("wqkvT", [D, QW + 2 * Dh], F32R, kind="ExternalInput").ap()
    woT_d = nc.dram_tensor("woT", [QW, D], F32R, kind="ExternalInput").ap()
    maskT_d = nc.dram_tensor("maskT", [S, S], F32, kind="ExternalInput").ap()
    ropeq_d = nc.dram_tensor("ropeq", [S, 2 * Dh], F32, kind="ExternalInput").ap()
    ropek_d = nc.dram_tensor("ropek", [S, 2 * Dh], F32, kind="ExternalInput").ap()
    outp_d = nc.dram_tensor("outp", [S, D], F32, kind="ExternalOutput").ap()
    kout_d = nc.dram_tensor("kout", [S, Dh], F32, kind="ExternalOutput").ap()
    vout_d = nc.dram_tensor("vout", [S, Dh], F32, kind="ExternalOutput").ap()

    with tile.TileContext(nc) as tc:
        with tc.tile_pool(name="glob", bufs=1) as glob:
            ident = glob.tile([128, 128], F32, tag="ident")
            make_identity(nc, ident[:])
            eps_t = glob.tile([128, 1], F32, tag="eps")
            nc.vector.memset(eps_t[:], EPS)
            qT = glob.tile([Dh, HG, S], F32R, tag="qT")
            kT = glob.tile([Dh, S], F32R, tag="kT")
            vv = glob.tile([128, KB, Dh + 1], F32R, tag="vv")
            attnT = glob.tile([Dh, HG, S], F32R, tag="attnT")
            woT = glob.tile([Dh, HG, D], F32R, tag="woT")
            nc.sync.dma_start(out=woT[:], in_=woT_d.rearrange("(h p) n -> p h n", p=Dh))
            one_c = nc.const_aps.tensor(1.0, (128, 1), F32)

            # ---- Phase 1: QKV projection + RMSNorm + RoPE + transposes ----
            with tc.tile_pool(name="ph1", bufs=1) as ph1, \
                 tc.tile_pool(name="hs_pool", bufs=3) as hs_pool, \
                 tc.tile_pool(name="sc1", bufs=2) as sc1, \
                 tc.tile_pool(name="ps1", bufs=2, space="PSUM") as ps1:
                w_sb = ph1.tile([128, D // 128, QW + 2 * Dh], F32R, tag="w_sb")
                nc.sync.dma_start(out=w_sb[:], in_=wqkvT_d.rearrange("(d p) n -> p d n", p=128))

                for s16 in range(S16):
                    sblk = slice(s16 * 128, (s16 + 1) * 128)
                    hst = hs_pool.tile([128, D // 128, 128], F32R, tag="hst")
                    nc.sync.dma_start(
                        out=hst[:], in_=hsT_d.rearrange("(d p) s -> p d s", p=128)[:, :, sblk])
                    rq = hs_pool.tile([128, 2 * Dh], F32, tag="rq")
                    rk = hs_pool.tile([128, 2 * Dh], F32, tag="rk")
                    nc.sync.dma_start(out=rq[:], in_=ropeq_d[sblk, :])
                    nc.sync.dma_start(out=rk[:], in_=ropek_d[sblk, :])

                    psA = ps1.tile([128, 512], F32, tag="psA")
                    psB = ps1.tile([128, 64], F32, tag="psB")
                    for d in range(D // 128):
                        nc.tensor.matmul(psA[:], hst[:, d, :], w_sb[:, d, 0:512], start=(d == 0), stop=(d == D // 128 - 1))
                    for d in range(D // 128):
                        nc.tensor.matmul(psB[:], hst[:, d, :], w_sb[:, d, 512:576], start=(d == 0), stop=(d == D // 128 - 1))
                    # layout: psA = [q(384) | k(96) | v(0:32)], psB = v(32:96)

                    # sum of squares per head (4 q heads + 1 k head) via ACT Square+accum
                    ssq = sc1.tile([128, HG + 1], F32, tag="ssq")
                    sqs = sc1.tile([128, Dh], F32, tag="sqs")
                    for h in range(HG):
                        nc.scalar.activation(sqs[:], psA[:, h * Dh:(h + 1) * Dh], AF.Square,
                                             accum_out=ssq[:, h:h + 1])
                    nc.scalar.activation(sqs[:], psA[:, QW:QW + Dh], AF.Square,
                                         accum_out=ssq[:, HG:HG + 1])
                    rinv = sc1.tile([128, HG + 1], F32, tag="rinv")
                    nc.scalar.activation(rinv[:], ssq[:], AF.Sqrt, scale=1.0 / Dh, bias=eps_t[:])
                    nc.vector.reciprocal(rinv[:], rinv[:])

                    # RoPE; rope tables have norm weight (and q scaling) folded in
                    qro = sc1.tile([128, QW], F32, tag="qro")
                    qsc = sc1.tile([128, Dh], F32, tag="qsc")
                    hd = Dh // 2
                    for h in range(HG):
                        qs = slice(h * Dh, (h + 1) * Dh)
                        r = rinv[:, h:h + 1]
                        nc.vector.scalar_tensor_tensor(
                            qro[:, qs], psA[:, qs], r, rq[:, 0:Dh], op0=ALU.mult, op1=ALU.mult)
                        nc.vector.scalar_tensor_tensor(
                            qsc[:, 0:hd], psA[:, h * Dh + hd:(h + 1) * Dh], r,
                            rq[:, Dh:Dh + hd], op0=ALU.mult, op1=ALU.mult)
                        nc.vector.scalar_tensor_tensor(
                            qsc[:, hd:Dh], psA[:, h * Dh:h * Dh + hd], r,
                            rq[:, Dh + hd:2 * Dh], op0=ALU.mult, op1=ALU.mult)
                        nc.vector.tensor_add(qro[:, qs], qro[:, qs], qsc[:])
                    kro = sc1.tile([128, Dh], F32, tag="kro")
                    ksc = sc1.tile([128, Dh], F32, tag="ksc")
                    ks = slice(QW, QW + Dh)
                    rk_r = rinv[:, HG:HG + 1]
                    nc.vector.scalar_tensor_tensor(
                        kro[:], psA[:, ks], rk_r, rk[:, 0:Dh], op0=ALU.mult, op1=ALU.mult)
                    nc.vector.scalar_tensor_tensor(
                        ksc[:, 0:hd], psA[:, QW + hd:QW + Dh], rk_r, rk[:, Dh:Dh + hd],
                        op0=ALU.mult, op1=ALU.mult)
                    nc.vector.scalar_tensor_tensor(
                        ksc[:, hd:Dh], psA[:, QW:QW + hd], rk_r, rk[:, Dh + hd:2 * Dh],
                        op0=ALU.mult, op1=ALU.mult)
                    nc.vector.tensor_add(kro[:], kro[:], ksc[:])

                    # v natural tiles (with trailing ones column preset)
                    vf = sc1.tile([128, Dh], F32, tag="vf")
                    nc.vector.tensor_copy(vf[:, 0:32], psA[:, QW + Dh:512])
                    nc.vector.tensor_copy(vf[:, 32:Dh], psB[:, 0:64])
                    nc.vector.tensor_copy(vv[:, s16, 0:Dh], vf[:])
                    nc.vector.tensor_copy(vv[:, s16, Dh:Dh + 1], one_c)

                    # outputs k, v
                    nc.sync.dma_start(out=kout_d[sblk, :], in_=kro[:])
                    nc.sync.dma_start(out=vout_d[sblk, :], in_=vf[:])

                    # transposes into [Dh, S] layouts
                    for h in range(HG):
                        trp = ps1.tile([Dh, 128], F32, tag="trp")
                        nc.tensor.transpose(trp[:], qro[:, h * Dh:(h + 1) * Dh], ident[:])
                        nc.vector.tensor_copy(qT[:, h, sblk], trp[:])
                    trk = ps1.tile([Dh, 128], F32, tag="trk")
                    nc.tensor.transpose(trk[:], kro[:], ident[:])
                    nc.vector.tensor_copy(kT[:, sblk], trk[:])

            # ---- Phase 3: attention (scores^T -> exp -> PV), causal-skipped ----
            with tc.tile_pool(name="mpool", bufs=2) as mpool, \
                 tc.tile_pool(name="epool", bufs=4) as epool, \
                 tc.tile_pool(name="rpool", bufs=2) as rpool, \
                 tc.tile_pool(name="ps3", bufs=2, space="PSUM") as ps3:
                for qb in range(QB):
                    qsl = slice(qb * 512, (qb + 1) * 512)
                    nkb = 4 * qb + 4
                    mt = mpool.tile([128, 4, 512], F32, tag="mt")
                    for j in range(4):
                        kb = 4 * qb + j
                        nc.sync.dma_start(
                            out=mt[:, j, :], in_=maskT_d[kb * 128:(kb + 1) * 128, qsl])
                    for h in range(HG):
                        po = ps3.tile([Dh + 1, 512], F32, tag="po")
                        for kb in range(nkb):
                            pss = ps3.tile([128, 512], F32, tag="pss")
                            nc.tensor.matmul(pss[:], kT[:, kb * 128:(kb + 1) * 128], qT[:, h, qsl], start=True, stop=True)
                            et = epool.tile([128, 512], F32R, tag="et")
                            if kb >= 4 * qb:
                                nc.vector.tensor_add(et[:], pss[:], mt[:, kb - 4 * qb, :])
                                nc.scalar.activation(et[:], et[:], AF.Exp)
                            else:
                                nc.scalar.activation(et[:], pss[:], AF.Exp)
                            nc.tensor.matmul(po[:], vv[:, kb, :], et[:], start=(kb == 0), stop=(kb == nkb - 1))
                        # normalize: attnT[:, h, qsl] = po[0:Dh] * (1 / po[Dh])
                        rb = rpool.tile([Dh + 1, 512], F32, tag="rb")
                        nc.vector.reciprocal(rb[Dh:Dh + 1, :], po[Dh:Dh + 1, :])
                        nc.gpsimd.partition_broadcast(rb[0:Dh, :], rb[Dh:Dh + 1, :])
                        nc.vector.tensor_mul(attnT[:, h, qsl], po[0:Dh, :], rb[0:Dh, :])

            # ---- Phase 4: partial out-projection ----
            with tc.tile_pool(name="opool", bufs=3) as opool, \
                 tc.tile_pool(name="ps4", bufs=4, space="PSUM") as ps4:
                for sq in range(S16):
                    for oc in range(D // 512):
                        pp = ps4.tile([128, 512], F32, tag="pp")
                        for h in range(HG):
                            nc.tensor.matmul(pp[:], attnT[:, h, sq * 128:(sq + 1) * 128],
                                             woT[:, h, oc * 512:(oc + 1) * 512],
                                             start=(h == 0), stop=(h == HG - 1))
                        ob = opool.tile([128, 512], F32, tag="ob")
                        nc.vector.tensor_copy(ob[:], pp[:])
                        nc.sync.dma_start(
                            out=outp_d[sq * 128:(sq + 1) * 128, oc * 512:(oc + 1) * 512],
                            in_=ob[:])

    nc.compile()
    return nc


_NC_CACHE = None


def _get_nc():
    global _NC_CACHE
    if _NC_CACHE is None:
        _NC_CACHE = build_program()
    return _NC_CACHE


def _prep_core_inputs(hidden_states, cos, sin, attention_mask, Wq, Wk, Wv, Wo,
                      q_norm_w, k_norm_w):
    hs = np.asarray(hidden_states, dtype=np.float32)
    cos = np.asarray(cos, dtype=np.float32)
    sin = np.asarray(sin, dtype=np.float32)
    mask = np.asarray(attention_mask, dtype=np.float32)[0, 0]
    Wq = np.asarray(Wq, dtype=np.float32)
    Wk = np.asarray(Wk, dtype=np.float32)
    Wv = np.asarray(Wv, dtype=np.float32)
    Wo = np.asarray(Wo, dtype=np.float32)
    qw = np.asarray(q_norm_w, dtype=np.float32)
    kw = np.asarray(k_norm_w, dtype=np.float32)

    maskT = np.ascontiguousarray(mask.T)
    hsT = [np.ascontiguousarray(hs[b].T) for b in range(B)]
    WqT, WkT, WvT, WoT = Wq.T, Wk.T, Wv.T, Wo.T  # (D, H*Dh), (D, KV*Dh), ..., (H*Dh, D)

    hd = Dh // 2

    def rope_tables(c, s, w, scale):
        # cols 0:96 -> w*cos*scale ; 96:144 -> -w[i+48]*sin[:, i]*scale (mult src qn[:,i+48])
        # 144:192 -> w[k]*sin[:, k+48]*scale (mult src qn[:,k], out col k+48)
        t = np.empty((S, 2 * Dh), dtype=np.float32)
        t[:, 0:Dh] = c * w[None, :] * scale
        t[:, Dh:Dh + hd] = -s[:, 0:hd] * w[None, hd:Dh] * scale
        t[:, Dh + hd:2 * Dh] = s[:, hd:Dh] * w[None, 0:hd] * scale
        return t

    in_maps = []
    for core in range(N_CORES):
        b, g = divmod(core, KV)
        wq_g = WqT[:, g * QW:(g + 1) * QW]          # (D, 384)
        wk_g = WkT[:, g * Dh:(g + 1) * Dh]          # (D, 96)
        wv_g = WvT[:, g * Dh:(g + 1) * Dh]
        wqkvT = np.ascontiguousarray(np.concatenate([wq_g, wk_g, wv_g], axis=1))
        woT_g = np.ascontiguousarray(WoT[g * QW:(g + 1) * QW, :])   # (384, D)
        in_maps.append({
            "hsT": hsT[b],
            "wqkvT": wqkvT,
            "woT": woT_g,
            "maskT": maskT,
            "ropeq": rope_tables(cos[b], sin[b], qw, SCALING),
            "ropek": rope_tables(cos[b], sin[b], kw, 1.0),
        })
    return in_maps


def kernel(hidden_states, cos, sin, attention_mask, Wq, Wk, Wv, Wo,
           q_norm_w, k_norm_w):
    nc = _get_nc()
    in_maps = _prep_core_inputs(hidden_states, cos, sin, attention_mask,
                                Wq, Wk, Wv, Wo, q_norm_w, k_norm_w)
    res = run_bass_kernel_spmd(nc, in_maps, core_ids=list(range(N_CORES)))
    out = np.zeros((B, S, D), dtype=np.float32)
    k = np.empty((B, KV, S, Dh), dtype=np.float32)
    v = np.empty((B, KV, S, Dh), dtype=np.float32)
    for core in range(N_CORES):
        b, g = divmod(core, KV)
        r = res.results[core]
        out[b] += r["outp"]
        k[b, g] = r["kout"]
        v[b, g] = r["vout"]
    return out, k, v
